# revision 4
# baseline (speedup 1.0000x reference)
"""Trainium2 Bass kernel for DeepCrossModalAlign.

Computation (per batch b):
  pa   = ppl @ Wp + bp                      [LP, H]
  ans  = answer @ Wa + ba                   [LA, H]
  s    = tanh(pa[:,None,:] + ans[None,:,:]) [LP, LA, H]
  sc   = s . ww + wb                        [LP, LA]
  attn = softmax(mask(sc), axis=-1)
  agg  = attn @ answer                      [LP, DA]
  out  = relu(concat(ppl, agg) @ Wo + bo)   [LP, DO]
returns (out, agg)

Sharding: data-parallel over batch B=16 across 8 cores (2 per core),
weights replicated. All GEMMs run in float32r (TF32-like, full PE rate).
The bahdanau cube runs on the scalar engine as tanh(paT + bias) with H on
partitions so the broadcast-add rides the free per-partition bias slot; the
ww-weighted reduction over H rides the tensor engine via a shifted-diagonal
stationary so scores for all 32 answers accumulate into one [32, 512] PSUM
bank. Softmax is computed max-free (|scores| <= sum|ww| ~ 18, exp cannot
overflow; masked entries multiply to exactly 0, matching the reference's
-1e8 fill after normalization).
"""
import sys

if "/opt/trn_rl_repo" not in sys.path:
    sys.path.insert(0, "/opt/trn_rl_repo")

import numpy as np

B, LP, LA = 16, 512, 32
DP, DA, H, DO = 1024, 512, 512, 1024
NCORES = 8
BL = B // NCORES  # batches per core
NDT = DP // 128   # 8  d-tiles of ppl/Wp
NHT = H // 128    # 4  h-tiles
NAT = DA // 128   # 4  d-tiles of answer/Wa
NPT = LP // 128   # 4  p-tiles
NCT = (DP + DA) // 128  # 12 c-tiles of concat/Wo

_cache = {}


def _build():
    import concourse.bacc as bacc
    import concourse.mybir as mybir
    import concourse.tile as tile

    f32 = mybir.dt.float32
    f32r = mybir.dt.float32r
    TANH = mybir.ActivationFunctionType.Tanh
    EXP = mybir.ActivationFunctionType.Exp

    nc = bacc.Bacc("TRN2", debug=False, num_devices=NCORES)

    # --- per-core inputs ---
    ppl_d = nc.dram_tensor("ppl", [BL, LP, DP], f32r, kind="ExternalInput")
    ans_d = nc.dram_tensor("answer", [BL, LA, DA], f32r, kind="ExternalInput")
    maskT_d = nc.dram_tensor("maskT", [LA, BL], f32, kind="ExternalInput")
    # --- replicated weights / constants ---
    Wp_d = nc.dram_tensor("Wp", [DP, H], f32r, kind="ExternalInput")
    Wa_d = nc.dram_tensor("Wa", [DA, H], f32r, kind="ExternalInput")
    Wo_d = nc.dram_tensor("Wo", [DP + DA, DO], f32r, kind="ExternalInput")
    bo_d = nc.dram_tensor("bo", [1, DO], f32r, kind="ExternalInput")
    bpba_d = nc.dram_tensor("bpba", [128, NHT], f32, kind="ExternalInput")
    wwdiag_d = nc.dram_tensor("wwdiag", [128, NHT, 63], f32r, kind="ExternalInput")
    ident_d = nc.dram_tensor("ident", [128, 128], f32r, kind="ExternalInput")
    ones_d = nc.dram_tensor("onesc", [LA, 128], f32r, kind="ExternalInput")
    # --- outputs ---
    out_d = nc.dram_tensor("aligned", [BL, LP, DO], f32, kind="ExternalOutput")
    agg_d = nc.dram_tensor("agg", [BL, LP, DA], f32, kind="ExternalOutput")

    with tile.TileContext(nc) as tc:
        with tc.tile_pool(name="persist", bufs=1) as P, \
             tc.tile_pool(name="pplnat", bufs=3) as PN, \
             tc.tile_pool(name="tpool", bufs=4) as TP, \
             tc.tile_pool(name="ostage", bufs=6) as OS, \
             tc.tile_pool(name="ps_tr", bufs=2, space="PSUM") as PS_TR, \
             tc.tile_pool(name="ps_big", bufs=2, space="PSUM") as PS_BIG, \
             tc.tile_pool(name="ps_sc", bufs=2, space="PSUM") as PS_SC, \
             tc.tile_pool(name="ps_sm", bufs=2, space="PSUM") as PS_SM:

            # ---------------- persistent SBUF ----------------
            Wp_sb = P.tile([128, NDT, H], f32r)
            Wa_sb = P.tile([128, NAT, H], f32r)
            Wo_sb = P.tile([128, NCT, DO], f32r)
            bo_sb = P.tile([1, DO], f32r)
            bpba_sb = P.tile([128, NHT], f32)
            wwdiag_sb = P.tile([128, NHT, 63], f32r)
            ident_sb = P.tile([128, 128], f32r)
            ones_sb = P.tile([LA, 128], f32r)
            maskT_sb = P.tile([LA, BL], f32)
            answer_sb = P.tile([LA, BL, DA], f32r)
            ansrT_sb = P.tile([128, NAT, BL * LA], f32r)
            pplT_sb = P.tile([128, BL, NDT, LP], f32r)
            pa_sb = P.tile([128, NHT, BL, LP], f32)
            ansb_sb = P.tile([128, NHT, BL * LA], f32)
            aggT_sb = P.tile([128, BL, NAT, LP], f32r)
            Em_sb = P.tile([LA, BL, LP], f32r)
            attnT_sb = P.tile([LA, BL, LP], f32r)
            r_sb = P.tile([128, BL, NPT], f32)
            rrow_sb = P.tile([1, BL, LP], f32r)

            # ---------------- constant / weight loads ----------------
            nc.sync.dma_start(out=ident_sb, in_=ident_d[:, :])
            nc.sync.dma_start(out=ones_sb, in_=ones_d[:, :])
            nc.sync.dma_start(out=wwdiag_sb, in_=wwdiag_d[:, :, :])
            nc.sync.dma_start(out=bpba_sb, in_=bpba_d[:, :])
            nc.sync.dma_start(out=maskT_sb, in_=maskT_d[:, :])
            nc.sync.dma_start(out=bo_sb, in_=bo_d[:, :])
            nc.sync.dma_start(
                out=Wp_sb, in_=Wp_d.ap().rearrange("(dt dc) h -> dc dt h", dc=128))
            nc.sync.dma_start(
                out=Wa_sb, in_=Wa_d.ap().rearrange("(dt dc) h -> dc dt h", dc=128))
            nc.sync.dma_start(
                out=Wo_sb, in_=Wo_d.ap().rearrange("(ct cc) o -> cc ct o", cc=128))
            for b in range(BL):
                nc.sync.dma_start(out=answer_sb[:, b, :], in_=ans_d[b, :, :])

            # ---------------- ppl transpose:  pplT[d, p] ----------------
            for b in range(BL):
                for pt in range(NPT):
                    pnat = PN.tile([128, DP], f32r, tag="pplnat")
                    nc.sync.dma_start(out=pnat, in_=ppl_d[b, pt * 128:(pt + 1) * 128, :])
                    for dt in range(NDT):
                        tp = PS_TR.tile([128, 128], f32r, tag="tr")
                        nc.tensor.transpose(tp, pnat[:, dt * 128:(dt + 1) * 128], ident_sb)
                        nc.vector.tensor_copy(
                            pplT_sb[:, b, dt, pt * 128:(pt + 1) * 128], tp)

            # ---------------- answer transpose: ansrT[d, (b,a)] ----------------
            for b in range(BL):
                for dt in range(NAT):
                    tp = PS_TR.tile([128, LA], f32r, tag="tr")
                    nc.tensor.transpose(
                        tp, answer_sb[:, b, dt * 128:(dt + 1) * 128],
                        ident_sb[0:LA, 0:LA])
                    nc.vector.tensor_copy(
                        ansrT_sb[:, dt, b * LA:(b + 1) * LA], tp)

            # ---------------- GEMM2: ansT = Wa.T @ ansrT  (+bias later) --------
            for ht in range(NHT):
                ps = PS_BIG.tile([128, BL * LA], f32, tag="big")
                for dt in range(NAT):
                    nc.tensor.matmul(
                        ps, Wa_sb[:, dt, ht * 128:(ht + 1) * 128],
                        ansrT_sb[:, dt, :],
                        start=(dt == 0), stop=(dt == NAT - 1))
                # ansb = ansT + (bp + ba)[h]
                nc.vector.tensor_scalar_add(
                    ansb_sb[:, ht, :], ps, bpba_sb[:, ht:ht + 1])

            # ---------------- GEMM1: paT[h, p] = Wp.T @ pplT ----------------
            for ht in range(NHT):
                for b in range(BL):
                    ps = PS_BIG.tile([128, LP], f32, tag="big")
                    for dt in range(NDT):
                        nc.tensor.matmul(
                            ps, Wp_sb[:, dt, ht * 128:(ht + 1) * 128],
                            pplT_sb[:, b, dt, :],
                            start=(dt == 0), stop=(dt == NDT - 1))
                    nc.vector.tensor_copy(pa_sb[:, ht, b, :], ps)

            # ---------------- main: tanh cube + score reduce + softmax --------
            def tanh_scores(b):
                sc_ps = PS_SC.tile([LA, LP], f32, tag="sc")
                for a in range(LA):
                    for ht in range(NHT):
                        t = TP.tile([128, LP], f32r, tag="t")
                        nc.scalar.activation(
                            t, pa_sb[:, ht, b, :], TANH,
                            bias=ansb_sb[:, ht, b * LA + a:b * LA + a + 1])
                        nc.tensor.matmul(
                            sc_ps, wwdiag_sb[:, ht, 31 - a:63 - a], t,
                            start=(a == 0 and ht == 0),
                            stop=(a == LA - 1 and ht == NHT - 1))
                return sc_ps

            def softmax_agg(b, sc_ps):
                # E = exp(scores) ; Em = E * mask
                E = OS.tile([LA, LP], f32, tag="E")
                nc.scalar.activation(E, sc_ps, EXP)
                nc.vector.tensor_scalar_mul(
                    Em_sb[:, b, :], E, maskT_sb[:, b:b + 1])
                # S[p] = sum_a Em -> [128, 128] blocks (columns replicated)
                S_ps = PS_SM.tile([128, LP], f32, tag="sm")
                for pt in range(NPT):
                    nc.tensor.matmul(
                        S_ps[:, pt * 128:(pt + 1) * 128],
                        Em_sb[:, b, pt * 128:(pt + 1) * 128],
                        ones_sb[:, 0:128],
                        start=(pt == 0), stop=(pt == NPT - 1))
                for pt in range(NPT):
                    nc.vector.reciprocal(
                        r_sb[:, b, pt:pt + 1], S_ps[:, pt * 128:pt * 128 + 1])
                # r as a row: transpose each [128,1] -> [1,128]
                rr_ps = PS_SM.tile([1, LP], f32, tag="sm")
                for pt in range(NPT):
                    nc.tensor.transpose(
                        rr_ps[:, pt * 128:(pt + 1) * 128],
                        r_sb[:, b, pt:pt + 1],
                        ident_sb.bitcast(f32))
                nc.vector.tensor_copy(rrow_sb[:, b, :], rr_ps)
                # R32 = broadcast r-row over 32 partitions
                R32_ps = PS_SM.tile([LA, LP], f32, tag="sm")
                nc.tensor.matmul(R32_ps, ones_sb[0:1, 0:LA], rrow_sb[:, b, :],
                                 start=True, stop=True)
                nc.vector.tensor_mul(attnT_sb[:, b, :], Em_sb[:, b, :], R32_ps)
                # agg[p, d] = (Em.T @ answer) * r   -> output
                for pt in range(NPT):
                    ag_ps = PS_BIG.tile([128, DA], f32, tag="big")
                    nc.tensor.matmul(
                        ag_ps, Em_sb[:, b, pt * 128:(pt + 1) * 128],
                        answer_sb[:, b, :], start=True, stop=True)
                    ao = OS.tile([128, DA], f32, tag="o")
                    nc.vector.tensor_scalar_mul(ao, ag_ps, r_sb[:, b, pt:pt + 1])
                    nc.sync.dma_start(
                        out=agg_d[b, pt * 128:(pt + 1) * 128, :], in_=ao)
                # aggT[d, p] = answer.T @ attnT  (stationary input to GEMM3)
                for dt in range(NAT):
                    at_ps = PS_BIG.tile([128, LP], f32, tag="big")
                    nc.tensor.matmul(
                        at_ps, answer_sb[:, b, dt * 128:(dt + 1) * 128],
                        attnT_sb[:, b, :], start=True, stop=True)
                    nc.vector.tensor_copy(aggT_sb[:, b, dt, :], at_ps)

            def gemm3(b):
                for pt in range(NPT):
                    psA = PS_BIG.tile([128, 512], f32, tag="big")
                    psB = PS_BIG.tile([128, 512], f32, tag="big")
                    for ct in range(NCT):
                        if ct < NDT:
                            lhsT = pplT_sb[:, b, ct, pt * 128:(pt + 1) * 128]
                        else:
                            lhsT = aggT_sb[:, b, ct - NDT, pt * 128:(pt + 1) * 128]
                        nc.tensor.matmul(psA, lhsT, Wo_sb[:, ct, 0:512],
                                         start=(ct == 0), stop=False)
                        nc.tensor.matmul(psB, lhsT, Wo_sb[:, ct, 512:1024],
                                         start=(ct == 0), stop=False)
                    nc.tensor.matmul(psA, ones_sb[0:1, :], bo_sb[:, 0:512],
                                     start=False, stop=True)
                    nc.tensor.matmul(psB, ones_sb[0:1, :], bo_sb[:, 512:1024],
                                     start=False, stop=True)
                    for nch, ps in ((0, psA), (1, psB)):
                        o = OS.tile([128, 512], f32, tag="o")
                        nc.vector.tensor_scalar_max(o, ps, 0.0)
                        nc.sync.dma_start(
                            out=out_d[b, pt * 128:(pt + 1) * 128,
                                      nch * 512:(nch + 1) * 512],
                            in_=o)

            sc0 = tanh_scores(0)
            softmax_agg(0, sc0)
            sc1 = tanh_scores(1)
            gemm3(0)
            softmax_agg(1, sc1)
            gemm3(1)

    nc.compile()
    return nc


def _get_nc():
    if "nc" not in _cache:
        _cache["nc"] = _build()
    return _cache["nc"]


def _host_prep(ppl, loc, answer, answer_mask, Wp, bp, Wa, ba, ww, wb, Wo, bo):
    ppl = np.ascontiguousarray(np.asarray(ppl, np.float32))
    answer = np.ascontiguousarray(np.asarray(answer, np.float32))
    mask = np.asarray(answer_mask).astype(np.float32)
    Wp = np.ascontiguousarray(np.asarray(Wp, np.float32))
    Wa = np.ascontiguousarray(np.asarray(Wa, np.float32))
    Wo = np.ascontiguousarray(np.asarray(Wo, np.float32))
    ww = np.asarray(ww, np.float32)
    bpba = (np.asarray(bp, np.float32) + np.asarray(ba, np.float32))
    # [128, NHT] with h = ht*128 + hc  ->  bpba_t[hc, ht]
    bpba_t = np.ascontiguousarray(bpba.reshape(NHT, 128).T)
    # shifted-diagonal ww: col 31 holds ww[ht*128 : (ht+1)*128]
    wwd = np.zeros((128, NHT, 63), np.float32)
    for ht in range(NHT):
        wwd[:, ht, 31] = ww[ht * 128:(ht + 1) * 128]
    ident = np.eye(128, dtype=np.float32)
    onesc = np.ones((LA, 128), np.float32)
    bo_r = np.asarray(bo, np.float32).reshape(1, DO)
    shared = {
        "Wp": Wp, "Wa": Wa, "Wo": Wo, "bo": np.ascontiguousarray(bo_r),
        "bpba": bpba_t, "wwdiag": wwd, "ident": ident, "onesc": onesc,
    }
    in_maps = []
    for c in range(NCORES):
        sl = slice(c * BL, (c + 1) * BL)
        m = dict(shared)
        m["ppl"] = np.ascontiguousarray(ppl[sl])
        m["answer"] = np.ascontiguousarray(answer[sl])
        m["maskT"] = np.ascontiguousarray(mask[sl].T)
        in_maps.append(m)
    return in_maps


def kernel(ppl, loc, answer, answer_mask, Wp, bp, Wa, ba, ww, wb, Wo, bo):
    from concourse.bass_utils import run_bass_kernel_spmd

    nc = _get_nc()
    in_maps = _host_prep(ppl, loc, answer, answer_mask, Wp, bp, Wa, ba,
                         ww, wb, Wo, bo)
    res = run_bass_kernel_spmd(nc, in_maps, core_ids=list(range(NCORES)))
    aligned = np.concatenate([res.results[c]["aligned"] for c in range(NCORES)], 0)
    agg = np.concatenate([res.results[c]["agg"] for c in range(NCORES)], 0)
    return aligned, agg


if __name__ == "__main__":
    nc = _get_nc()
    print("build + compile OK")
